# revision 83
# baseline (speedup 1.0000x reference)
"""MiMoV2 sparse attention (GQA + sliding window + sink) on 8 TRN2 cores.

Tensor-parallel over heads: core c owns q heads 4c..4c+3 and kv head c
(GQA groups align with cores); host sums the 8 partial o_proj outputs.

Per-core dataflow, software-pipelined so the PE never idles:
  KV phase: kT/vT projections for all 4 token tiles (bf16 weights/h),
    RoPE on k, v transposed to [tok, d] via PE.
  Slots n=0..5: Q-proj(n) + attention(n-1) + o_proj(n-2), with the three
    instruction streams round-robin interleaved at ~1us granularity so
    exp/activation latency hides under projection/o_proj matmuls.

Attention per (head, 512-query tile), column-sliced to the visible
  window: each key subtile ks only computes the query columns [c0,c1)
  that can see it (108 [128x128] units instead of 144), cutting score
  and AV matmul work by 25%. S^T[k,q] = kT.T @ qT over [c0:c1) (bf16 in,
  f32 psum); w = exp(S^T) bf16 on Act (AV deferred one chunk to hide exp
  latency); only the diagonal/window-edge 128x128 blocks get 0/1 masks
  (2 patterns, DVE); attnT accumulated via per-region PSUM writes (the
  first matmul's start=True pending-zeroes the whole bank; later
  matmuls split at the coverage boundary so each write region is
  uniformly fresh or accumulating); wsum += w per region (DVE);
  denom = ones.T @ wsum (one PE matmul, issued before the final AV so
  the DVE add/recip chain overlaps it) + exp(sink); 1/denom broadcast
  via GpSimd; attnT *= rbc.
o_proj in transposed layout: outT[oc, tok] += woT.T @ attnT, written
  bf16 to a packed DRAM tensor; host sums partials and transposes (free).
  For the last tile the first chunk group pre-runs its hd0-2 matmuls
  across four banks so the last head's softmax finalize overlaps o_proj.

Startup DMA: the dynamic HW queues race fairly for HBM, so transfers
are split to consumption order and later-needed tensors (wq tails,
cos/sin tails, masks, wo) are gated behind scalar-engine ops whose
data deps release them exactly when slot 0 stops needing the bandwidth.
h tiles live in 5 rotating buffers (tags (4n+q)%5) so each slot's first
h load is not WAR-blocked on the previous slot's Q-pass reads.

Softmax uses a constant (zero) max-shift: scores are bounded (|s| < ~10)
far below fp32/bf16 exp overflow, and softmax is shift-invariant; the
sink logit enters the denominator as exp(sink).
"""
import os
import numpy as np
import ml_dtypes

import concourse.mybir as mybir
import concourse.tile as tile
from concourse import bacc
from concourse.bass_utils import run_bass_kernel_spmd
from contextlib import ExitStack

F32 = mybir.dt.float32
BF16 = mybir.dt.bfloat16

S = 2048
HID = 4096
NQ = 32
NKV = 8
D = 128
WINDOW = 1024
THETA = 1e6
CORES = 8
QH = NQ // CORES          # 4 q heads per core
DQ = QH * D               # 512
NT = S // 512             # 4 token tiles of 512
KS = S // 128             # 16 key subtiles of 128

KT_DT = BF16

last_results = None       # set by kernel(); test.py reads exec_time_ns


def _schedule(positions):
    """Static attention schedule from the actual positions array.

    Returns (masks_np [128, P*128] bf16, sched, n_patterns) where
    sched[qt] = list of (ks, c0, c1, mask_ops); [c0,c1) is the visible
    query-column range within the tile (128-aligned) and mask_ops is a
    list of (col_offset, pidx) for partially-visible 128x128 blocks.
    """
    pos = np.asarray(positions).astype(np.int64)
    vis = (pos[None, :] <= pos[:, None]) & (pos[:, None] - pos[None, :] < WINDOW)
    patterns = {}
    plist = []
    sched = []
    for qt in range(NT):
        row = []
        for ks in range(KS):
            sub = vis[qt * 512:(qt + 1) * 512, ks * 128:(ks + 1) * 128]  # [q, k]
            qcols = sub.any(axis=1)
            if not qcols.any():
                continue
            c0 = int(np.argmax(qcols))
            c1 = 512 - int(np.argmax(qcols[::-1]))
            assert qcols[c0:c1].all(), "visible query range must be contiguous"
            c0 = (c0 // 128) * 128
            c1 = ((c1 + 127) // 128) * 128
            mops = []
            for u in range(c0, c1, 128):
                blk = sub[u:u + 128, :]
                if not blk.all():
                    pat = np.ascontiguousarray(blk.T).astype(np.float32)  # [k, q]
                    key = pat.tobytes()
                    if key not in patterns:
                        patterns[key] = len(plist)
                        plist.append(pat)
                    mops.append((u, patterns[key]))
            row.append((ks, c0, c1, mops))
        sched.append(row)
    if not plist:
        plist = [np.ones((128, 128), np.float32)]
    masks = np.concatenate(plist, axis=1).astype(ml_dtypes.bfloat16)  # [128, P*128]
    return masks, sched, len(plist)


def _build(sched, n_patterns):
    nc = bacc.Bacc("TRN2", target_bir_lowering=False)

    # Host-packed inputs: every load is a contiguous [128, n] slab.
    HB = nc.dram_tensor("hb", [128, NT * 4 * 8 * 512], BF16, kind="ExternalInput")
    WKV = nc.dram_tensor("wkv", [128, 2 * 32 * 128], BF16, kind="ExternalInput")
    WQa = nc.dram_tensor("wqa", [128, 32 * 256], BF16, kind="ExternalInput")
    WQb = nc.dram_tensor("wqb", [128, 32 * 256], BF16, kind="ExternalInput")
    WOT = nc.dram_tensor("wot", [128, QH * 32 * 128], BF16, kind="ExternalInput")
    Cos = nc.dram_tensor("cos", [128, S], F32, kind="ExternalInput")
    Sin = nc.dram_tensor("sin", [128, S], F32, kind="ExternalInput")
    Mk = nc.dram_tensor("mk", [128, n_patterns * 128], BF16, kind="ExternalInput")
    One = nc.dram_tensor("one", [128, 1], BF16, kind="ExternalInput")
    Esk = nc.dram_tensor("esk", [128, QH], F32, kind="ExternalInput")
    # packed partial output: cols = qt*16384 + o*512 + t  (o = hid/128 chunk)
    OutP = nc.dram_tensor("outp", [128, NT * 32 * 512], BF16, kind="ExternalOutput")

    Exp = mybir.ActivationFunctionType.Exp

    with tile.TileContext(nc) as tc, ExitStack() as top:
        persist = top.enter_context(tc.tile_pool(name="persist", bufs=1))
        ones = persist.tile([128, 1], BF16)
        esk = persist.tile([128, QH], F32)
        cos_sb = persist.tile([128, S], F32)
        sin_sb = persist.tile([128, S], F32)
        mk_sb = persist.tile([128, n_patterns * 128], BF16)
        wot_sb = persist.tile([128, QH * 32 * 128], BF16)
        wq_sb = [persist.tile([128, 32 * 256], BF16, tag=f"wq{i}", name=f"wq{i}")
                 for i in range(2)]
        wkv_sb = persist.tile([128, 2 * 32 * 128], BF16)
        # qT[m][n] is dead once b_stream(n) finishes (slot n+1), and
        # proj(n+2) rewrites the same parity only in slot n+2 — so two
        # buffers per head suffice (saves 8KB/partition of SBUF)
        qT_ = [[persist.tile([128, 512], BF16, tag=f"qT{m}_{p_}", name=f"qT{m}_{p_}")
                for p_ in range(2)] for m in range(QH)]
        qT = [[qT_[m][n % 2] for n in range(NT)] for m in range(QH)]
        kT = [persist.tile([128, 512], KT_DT, tag=f"kT{n}", name=f"kT{n}") for n in range(NT)]
        v_sb = [persist.tile([128, 512], BF16, tag=f"v{n}", name=f"v{n}") for n in range(NT)]

        def rope(dst, ps, n):
            co = cos_sb[:, n * 512:(n + 1) * 512]
            si = sin_sb[:, n * 512:(n + 1) * 512]
            t2 = rtmp.tile([128, 512], F32, tag="t2", name="t2")
            nc.vector.tensor_mul(t2[0:64, :], ps[64:128, :], si[0:64, :])
            nc.vector.tensor_mul(t2[64:128, :], ps[0:64, :], si[64:128, :])
            tc_ = rtmp.tile([128, 512], F32, tag="tc", name="tc")
            nc.vector.tensor_mul(tc_[:], ps[:], co)
            nc.vector.tensor_add(dst, tc_[:], t2[:])

        # ------- Slots: proj(n) [kv + q passes] + attn(n-1) + o_proj(n-2)
        with ExitStack() as pq:
            hqp = pq.enter_context(tc.tile_pool(name="hqp", bufs=1))
            rtmp = pq.enter_context(tc.tile_pool(name="rtmpq", bufs=2))
            vtp = pq.enter_context(tc.tile_pool(name="vtp", bufs=2))
            wpool = pq.enter_context(tc.tile_pool(name="wpool", bufs=7))
            wspool = pq.enter_context(tc.tile_pool(name="wspool", bufs=2))
            dpool = pq.enter_context(tc.tile_pool(name="dpool", bufs=2))
            apool = pq.enter_context(tc.tile_pool(name="apool", bufs=1))
            obp = pq.enter_context(tc.tile_pool(name="obp", bufs=2))
            ps_q = pq.enter_context(tc.tile_pool(name="ps_q", bufs=1, space="PSUM"))
            ps_s = pq.enter_context(tc.tile_pool(name="ps_s", bufs=2, space="PSUM"))
            ps_a = pq.enter_context(tc.tile_pool(name="ps_a", bufs=2, space="PSUM"))
            ps_o = pq.enter_context(tc.tile_pool(name="ps_o", bufs=2, space="PSUM"))

            attnT = [[None] * QH for _ in range(NT)]

            # ---- Startup DMA.  The dynamic HW queues race (fair-share
            # bandwidth), so prioritization = eager-issue only what slot 0
            # needs (h0 + wkv + cos0 + wqa ~ 8.5MB) and GATE everything else
            # behind the vt copy, whose KV-pass dependency releases it
            # exactly when the Q passes begin (see proj_stream).
            # h lives in 5 rotating whole tiles of [128, 4096] (tags
            # hq0-hq4); slot n uses tags (4n+q)%5 so each slot's first load
            # hits a fresh buffer and the rest WAR-release progressively as
            # the previous slot's Q passes retire their reads.
            def hq_tiles(n):
                out = []
                for q in range(4):
                    t_ = (4 * n + q) % 5
                    out.append(hqp.tile([128, 4096], BF16, tag=f"hq{t_}",
                                        name=f"hq{t_}"))
                return out

            hq0 = hq_tiles(0)
            # Anchor-laddered startup: the dynamic DMA queues share HBM
            # fairly, so an all-at-once issue makes everything arrive
            # together and late.  Instead each consumption stage's
            # transfers trigger behind a tiny scalar op that depends on the
            # previous h tile's arrival, keeping arrival order ~= use order
            # at near-full per-stage bandwidth.
            nc.sync.dma_start(hq0[0][:, 0:1024], HB[:, 0:1024])
            nc.sync.dma_start(hq0[0][:, 1024:2048], HB[:, 1024:2048])
            nc.sync.dma_start(hq0[0][:, 2048:4096], HB[:, 2048:4096])
            nc.sync.dma_start(cos_sb[:, 0:512], Cos[:, 0:512])
            nc.sync.dma_start(sin_sb[:, 0:512], Sin[:, 0:512])
            nc.scalar.dma_start(wkv_sb[:, 0:1024], WKV[:, 0:1024])
            nc.scalar.dma_start(wkv_sb[:, 4096:5120], WKV[:, 4096:5120])
            nc.gpsimd.dma_start(esk[:], Esk[:])
            nc.gpsimd.dma_start(ones[:], One[:])

            def ladder(stage, src_tile, anchor_col, dmas):
                # anchor on an early piece so the next stage joins the race
                # before the tile fully lands
                a_ = dpool.tile([1, 1], BF16, tag="dum", name=f"anc{stage}")
                nc.scalar.copy(a_[:], src_tile[0:1, anchor_col:anchor_col + 1])
                for dst, src in dmas:
                    nc.scalar.dma_start(dst, src)

            ladder(0, hq0[0], 2047, [
                (hq0[1][:, 0:2048], HB[:, 4096:6144]),
                (hq0[1][:, 2048:4096], HB[:, 6144:8192]),
                (wkv_sb[:, 1024:2048], WKV[:, 1024:2048]),
                (wkv_sb[:, 5120:6144], WKV[:, 5120:6144]),
                (wq_sb[0][:, 0:2048], WQa[:, 0:2048]),
            ])
            ladder(1, hq0[1], 2047, [
                (hq0[2][:, 0:2048], HB[:, 8192:10240]),
                (hq0[2][:, 2048:4096], HB[:, 10240:12288]),
                (wkv_sb[:, 2048:3072], WKV[:, 2048:3072]),
                (wkv_sb[:, 6144:7168], WKV[:, 6144:7168]),
                (wq_sb[0][:, 2048:4096], WQa[:, 2048:4096]),
            ])
            ladder(2, hq0[2], 2047, [
                (hq0[3][:, 0:2048], HB[:, 12288:14336]),
                (hq0[3][:, 2048:4096], HB[:, 14336:16384]),
                (wkv_sb[:, 3072:4096], WKV[:, 3072:4096]),
                (wkv_sb[:, 7168:8192], WKV[:, 7168:8192]),
                (wq_sb[0][:, 4096:8192], WQa[:, 4096:8192]),
            ])
            ladder(3, hq0[3], 2047, [
                (wq_sb[1][:, 0:2048], WQb[:, 0:2048]),
                (wq_sb[1][:, 2048:8192], WQb[:, 2048:8192]),
            ])

            hq_cache = {0: hq0}

            def qpass(n, mp, pool_tags):
                hqs = hq_cache[n]
                qps = [pool_tags[0].tile([128, 512], F32, tag=pool_tags[1][j],
                                         name=f"qps{j}")
                       for j in range(2)]
                for q in range(4):
                    for j in range(2):
                        for half in range(2):
                            for kk in range(half * 4, half * 4 + 4):
                                nc.tensor.matmul(
                                    qps[j][:],
                                    wq_sb[mp][:, (q * 8 + kk) * 256 + j * 128:
                                            (q * 8 + kk) * 256 + (j + 1) * 128],
                                    hqs[q][:, kk * 512:(kk + 1) * 512],
                                    start=(q == 0 and kk == 0),
                                    stop=(q == 3 and kk == 7))
                            yield
                for j in range(2):
                    rope(qT[mp * 2 + j][n][:], qps[j][:], n)
                    yield

            def proj_stream(n, mps=(0, 1)):
                if n == 0:
                    hqs = hq0
                    for _ in range(4):
                        yield
                else:
                    hqs = hq_tiles(n)
                    hq_cache[n] = hqs
                    for q in range(4):
                        nc.sync.dma_start(hqs[q][:],
                                          HB[:, (n * 4 + q) * 4096:(n * 4 + q + 1) * 4096])
                        yield

                def hcol(q, kk):
                    return hqs[q][:, kk * 512:(kk + 1) * 512]
                # pass 1: k and v projections
                kvps = [ps_q.tile([128, 512], F32, tag=f"proj{j}", name=f"proj{j}")
                        for j in range(2)]
                for q in range(4):
                    for m in range(2):
                        for half in range(2):
                            for kk in range(half * 4, half * 4 + 4):
                                nc.tensor.matmul(
                                    kvps[m][:],
                                    wkv_sb[:, m * 4096 + (q * 8 + kk) * 128:
                                           m * 4096 + (q * 8 + kk + 1) * 128],
                                    hcol(q, kk),
                                    start=(q == 0 and kk == 0), stop=(q == 3 and kk == 7))
                            yield
                rope(kT[n][:], kvps[0][:], n)
                yield
                vt = vtp.tile([128, 512], BF16, tag="vt", name=f"vt{n}")
                nc.scalar.copy(vt[:], kvps[1][:])
                if n == 0:
                    # gated prefetch: the scalar engine reaches these triggers
                    # only once the vt copy's KV-pass dependency clears, so
                    # none of it races the slot-0 critical loads
                    nc.scalar.dma_start(cos_sb[:, 512:2048], Cos[:, 512:2048])
                    nc.scalar.dma_start(sin_sb[:, 512:2048], Sin[:, 512:2048])
                    nc.scalar.dma_start(mk_sb[:], Mk[:])
                for t in range(4):
                    nc.sync.dma_start_transpose(v_sb[n][:, t * 128:(t + 1) * 128],
                                                vt[:, t * 128:(t + 1) * 128])
                yield
                # passes 2/3: q heads in pairs, reusing the same two banks
                for mp in mps:
                    yield from qpass(n, mp, (ps_q, ("proj0", "proj1")))

            def b_stream(qt):
                row = sched[qt]

                for hd in range(QH):
                    a_ps = ps_a.tile([128, 512], F32, tag="a")
                    wsum = wspool.tile([128, 512], BF16, tag="ws")
                    cov = 0       # wsum coverage: always [0, cov)
                    acov = [0]    # a_ps coverage (AV is deferred one chunk)

                    def do_av(w, ks, c0, c1, last):
                        vst = v_sb[ks // 4][:, (ks % 4) * 128:(ks % 4 + 1) * 128]
                        regs = []
                        if acov[0] > c0:
                            regs.append((c0, min(c1, acov[0]), False))
                        if c1 > acov[0]:
                            regs.append((max(c0, acov[0]), c1, acov[0] == 0))
                        for ri, (lo, hi, st) in enumerate(regs):
                            nc.tensor.matmul(
                                a_ps[:, lo:hi], vst, w[:, lo:hi],
                                start=st, stop=(last and ri == len(regs) - 1))
                        acov[0] = max(acov[0], c1)

                    pend = None  # (w, ks, c0, c1): AV matmul deferred one chunk
                    for i, (ks, c0, c1, mops) in enumerate(row):
                        s_ps = ps_s.tile([128, 512], F32, tag="s")
                        nc.tensor.matmul(
                            s_ps[:, c0:c1],
                            kT[ks // 4][:, (ks % 4) * 128:(ks % 4 + 1) * 128],
                            qT[hd][qt][:, c0:c1], start=True, stop=True)
                        if pend is not None:
                            do_av(*pend, last=False)
                        w = wpool.tile([128, 512], BF16, tag="w")
                        nc.scalar.activation(w[:, c0:c1], s_ps[:, c0:c1], Exp)
                        for (u, pidx) in mops:
                            nc.vector.tensor_mul(
                                w[:, u:u + 128], w[:, u:u + 128],
                                mk_sb[:, pidx * 128:(pidx + 1) * 128])
                        hi_old = min(c1, cov)
                        if hi_old > c0:
                            nc.vector.tensor_add(wsum[:, c0:hi_old],
                                                 wsum[:, c0:hi_old], w[:, c0:hi_old])
                        if c1 > cov:
                            nc.vector.tensor_copy(wsum[:, cov:c1], w[:, cov:c1])
                            cov = c1
                        pend = (w, ks, c0, c1)
                        yield
                    # denom first (needs only wsum) so the add/reciprocal run
                    # under the final AV matmul
                    d_ps = ps_s.tile([128, 512], F32, tag="s")
                    nc.tensor.matmul(d_ps[0:1, :], ones[:], wsum[:],
                                     start=True, stop=True)
                    do_av(*pend, last=True)
                    den = dpool.tile([1, 512], F32, tag="den")
                    nc.vector.tensor_scalar_add(den[:], d_ps[0:1, :], esk[0:1, hd:hd + 1])
                    rec = dpool.tile([1, 512], F32, tag="rec")
                    nc.vector.reciprocal_approx_fast(rec[:], den[:])
                    rbc = dpool.tile([128, 512], F32, tag="rbc")
                    nc.gpsimd.partition_broadcast(rbc[:], rec[:])
                    if qt == 0 and hd == 0:
                        # deferred bulk prefetch: the gpsimd engine reaches
                        # this trigger mid-slot-1, after the h1 race clears
                        nc.gpsimd.dma_start(wot_sb[:], WOT[:])
                    at = apool.tile([128, 512], BF16, tag=f"at{hd}_{qt % 2}")
                    nc.vector.tensor_mul(at[:], a_ps[:], rbc[:])
                    attnT[qt][hd] = at
                    yield

            def c_stream(qt):
                # the last tile runs with no other stream to hide the PSUM
                # drain, so borrow the idle proj banks for 4-deep pipelining
                deep = qt == NT - 1
                for og in range(8):
                    ob = obp.tile([128, 2048], BF16, tag="ob")
                    if deep and og == 0:
                        # prestart: run hd0-2 for SIX chunks (og0 c0-3 plus
                        # og1 c0-1, borrowing the idle ps_s/ps_a buffers)
                        # while the last head's softmax finalize (DVE/GpSimd
                        # chain, ~3.5us) produces attnT[qt][3].  The greedy
                        # scheduler drains independent PE work before the
                        # chain starts, so there must be more of it than the
                        # preceding score phase can absorb.
                        # the 7th chunk reuses d_ps's buffer: its WAR dep on
                        # the chain's denominator read makes it available
                        # only mid-chain, so the scheduler cannot hoist it
                        pre = []
                        for ci in range(7):
                            og_, c = divmod(ci, 4)
                            if ci in (4, 6):
                                o_ps = ps_s.tile([128, 512], F32, tag="s",
                                                 name="o_ps")
                            elif ci == 5:
                                o_ps = ps_a.tile([128, 512], F32, tag="a",
                                                 name="o_ps")
                            elif c % 2:
                                o_ps = ps_q.tile([128, 512], F32,
                                                 tag=f"proj{(c // 2) % 2}", name="o_ps")
                            else:
                                o_ps = ps_o.tile([128, 512], F32, tag="o")
                            pre.append(o_ps)
                            o = og_ * 4 + c
                            for hd in range(QH - 1):
                                nc.tensor.matmul(
                                    o_ps[:],
                                    wot_sb[:, (hd * 32 + o) * 128:(hd * 32 + o + 1) * 128],
                                    attnT[qt][hd][:],
                                    start=(hd == 0), stop=False)
                        yield
                        hd = QH - 1
                        for c in range(4):
                            nc.tensor.matmul(
                                pre[c][:],
                                wot_sb[:, (hd * 32 + c) * 128:(hd * 32 + c + 1) * 128],
                                attnT[qt][hd][:], start=False, stop=True)
                            if c % 2:
                                nc.scalar.copy(ob[:, c * 512:(c + 1) * 512], pre[c][:])
                            else:
                                nc.vector.tensor_copy(ob[:, c * 512:(c + 1) * 512],
                                                      pre[c][:])
                            yield
                        nc.gpsimd.dma_start(OutP[:, qt * 16384:qt * 16384 + 2048], ob[:])
                        yield
                        continue
                    if deep and og == 1:
                        for c in range(3):
                            o = 4 + c
                            nc.tensor.matmul(
                                pre[4 + c][:],
                                wot_sb[:, ((QH - 1) * 32 + o) * 128:
                                       ((QH - 1) * 32 + o + 1) * 128],
                                attnT[qt][QH - 1][:], start=False, stop=True)
                            if c % 2:
                                nc.scalar.copy(ob[:, c * 512:(c + 1) * 512], pre[4 + c][:])
                            else:
                                nc.vector.tensor_copy(ob[:, c * 512:(c + 1) * 512],
                                                      pre[4 + c][:])
                            yield
                        for c in range(3, 4):
                            o = og * 4 + c
                            o_ps = ps_q.tile([128, 512], F32, tag=f"proj{og % 2}",
                                             name="o_ps") if c % 2 else \
                                ps_o.tile([128, 512], F32, tag="o", name="o_ps")
                            for hd in range(QH):
                                nc.tensor.matmul(
                                    o_ps[:],
                                    wot_sb[:, (hd * 32 + o) * 128:(hd * 32 + o + 1) * 128],
                                    attnT[qt][hd][:],
                                    start=(hd == 0), stop=(hd == QH - 1))
                            if c % 2:
                                nc.scalar.copy(ob[:, c * 512:(c + 1) * 512], o_ps[:])
                            else:
                                nc.vector.tensor_copy(ob[:, c * 512:(c + 1) * 512], o_ps[:])
                            yield
                        nc.gpsimd.dma_start(
                            OutP[:, qt * 16384 + 2048:qt * 16384 + 4096], ob[:])
                        yield
                        continue
                    for c in range(4):
                        o = og * 4 + c
                        if deep and c % 2:
                            o_ps = ps_q.tile([128, 512], F32, tag=f"proj{og % 2}",
                                             name="o_ps")
                        else:
                            o_ps = ps_o.tile([128, 512], F32, tag="o")
                        for hd in range(QH):
                            nc.tensor.matmul(
                                o_ps[:],
                                wot_sb[:, (hd * 32 + o) * 128:(hd * 32 + o + 1) * 128],
                                attnT[qt][hd][:],
                                start=(hd == 0), stop=(hd == QH - 1))
                        if c % 2:
                            nc.scalar.copy(ob[:, c * 512:(c + 1) * 512], o_ps[:])
                        else:
                            nc.vector.tensor_copy(ob[:, c * 512:(c + 1) * 512], o_ps[:])
                        if deep and og == 7 and c == 1:
                            # start draining the final chunk group early so
                            # the tail DMA overlaps the last two chunks
                            nc.gpsimd.dma_start(
                                OutP[:, qt * 16384 + og * 2048:
                                     qt * 16384 + og * 2048 + 1024],
                                ob[:, 0:1024])
                        yield
                    if deep and og == 7:
                        nc.gpsimd.dma_start(
                            OutP[:, qt * 16384 + og * 2048 + 1024:
                                 qt * 16384 + (og + 1) * 2048],
                            ob[:, 1024:2048])
                    else:
                        nc.gpsimd.dma_start(
                            OutP[:, qt * 16384 + og * 2048:qt * 16384 + (og + 1) * 2048],
                            ob[:])
                    yield

            for slot in range(NT + 2):
                streams = []
                if slot == 0:
                    # slot 0 is DMA-paced: run KV and both Q passes as
                    # parallel streams on disjoint psum banks (ps_o/ps_s are
                    # idle here) so Q matmuls for already-arrived h tiles
                    # fill the KV pass's HBM stalls
                    streams.append(proj_stream(0, mps=()))
                    streams.append(qpass(0, 0, (ps_o, ("o", "o"))))
                    streams.append(qpass(0, 1, (ps_s, ("s", "s"))))
                elif slot < NT:
                    streams.append(proj_stream(slot))
                if 0 <= slot - 1 < NT:
                    streams.append(b_stream(slot - 1))
                if 0 <= slot - 2 < NT:
                    streams.append(c_stream(slot - 2))
                while streams:
                    alive = []
                    for st in streams:
                        try:
                            next(st)
                            alive.append(st)
                        except StopIteration:
                            pass
                    streams = alive

    nc.compile()
    return nc


def kernel(hidden_states, positions, wq, wk, wv, wo, sink):
    global last_results
    h = np.asarray(hidden_states, np.float32)
    pos = np.asarray(positions)
    wq = np.asarray(wq, np.float32)
    wk = np.asarray(wk, np.float32)
    wv = np.asarray(wv, np.float32)
    wo = np.asarray(wo, np.float32)
    sink = np.asarray(sink, np.float32)

    masks, sched, n_pat = _schedule(pos)
    nc = _build(sched, n_pat)

    # h packed: [p, (n, q, kt, t)] = h[n*512+t, (q*8+kt)*128+p]
    hp = np.ascontiguousarray(
        h.reshape(NT, 512, 4, 8, 128).transpose(4, 0, 2, 3, 1).reshape(128, -1)
    ).astype(ml_dtypes.bfloat16)

    # RoPE tables (neox half-split), rows duplicated for both halves
    inv_freq = 1.0 / (THETA ** (np.arange(0, D, 2, dtype=np.float64) / D))
    freqs = pos.astype(np.float64)[:, None] * inv_freq[None, :]       # [S, 64]
    cos = np.cos(freqs).astype(np.float32).T                          # [64, S]
    sin = np.sin(freqs).astype(np.float32).T
    cos_full = np.ascontiguousarray(np.concatenate([cos, cos], axis=0))
    sin_sign = np.ascontiguousarray(np.concatenate([-sin, sin], axis=0))

    scale = np.float32(D ** -0.5)
    esink = np.exp(sink.astype(np.float64)).astype(np.float32)

    in_maps = []
    for c in range(CORES):
        wqc = (wq[:, c * DQ:(c + 1) * DQ] * scale)                    # [HID, 512]
        # [p, kt, m*128+i] = wqc[kt*128+p, :], split into head pairs
        wqr = wqc.reshape(32, 128, 512).transpose(1, 0, 2)
        wqa = np.ascontiguousarray(wqr[:, :, 0:256].reshape(128, -1)).astype(ml_dtypes.bfloat16)
        wqb = np.ascontiguousarray(wqr[:, :, 256:512].reshape(128, -1)).astype(ml_dtypes.bfloat16)
        wkc = wk[:, c * D:(c + 1) * D].reshape(32, 128, 128).transpose(1, 0, 2)
        wvc = wv[:, c * D:(c + 1) * D].reshape(32, 128, 128).transpose(1, 0, 2)
        wkvp = np.ascontiguousarray(
            np.concatenate([wkc.reshape(128, -1), wvc.reshape(128, -1)], axis=1)
        ).astype(ml_dtypes.bfloat16)
        woc = wo[c * DQ:(c + 1) * DQ, :]                              # [512, HID]
        # [p, hd, o, i] = woc[hd*128+p, o*128+i]
        wotp = np.ascontiguousarray(
            woc.reshape(QH, 128, 32, 128).transpose(1, 0, 2, 3).reshape(128, -1)
        ).astype(ml_dtypes.bfloat16)
        in_maps.append({
            "hb": hp,
            "wqa": wqa,
            "wqb": wqb,
            "wkv": wkvp,
            "wot": wotp,
            "cos": cos_full,
            "sin": sin_sign,
            "mk": masks,
            "one": np.ones((128, 1), np.float32).astype(ml_dtypes.bfloat16),
            "esk": np.ascontiguousarray(
                np.broadcast_to(esink[None, c * QH:(c + 1) * QH], (128, QH)).copy()),
        })

    trace = bool(int(os.environ.get("KERNEL_TRACE", "0")))
    res = run_bass_kernel_spmd(nc, in_maps, core_ids=list(range(CORES)), trace=trace)
    last_results = res
    acc = np.zeros((128, NT * 32 * 512), np.float64)
    for r in res.results:
        acc += r["outp"].astype(np.float64)
    # [p, qt, o, t] -> out[qt*512+t, o*128+p]
    out = acc.reshape(128, NT, 32, 512).transpose(1, 3, 2, 0).reshape(S, HID)
    return out.astype(np.float32)


# revision 84
# speedup vs baseline: 1.0155x; 1.0155x over previous
"""MiMoV2 sparse attention (GQA + sliding window + sink) on 8 TRN2 cores.

Tensor-parallel over heads: core c owns q heads 4c..4c+3 and kv head c
(GQA groups align with cores); host sums the 8 partial o_proj outputs.

Per-core dataflow, software-pipelined so the PE never idles:
  KV phase: kT/vT projections for all 4 token tiles (bf16 weights/h),
    RoPE on k, v transposed to [tok, d] via PE.
  Slots n=0..5: Q-proj(n) + attention(n-1) + o_proj(n-2), with the three
    instruction streams round-robin interleaved at ~1us granularity so
    exp/activation latency hides under projection/o_proj matmuls.

Attention per (head, 512-query tile), column-sliced to the visible
  window: each key subtile ks only computes the query columns [c0,c1)
  that can see it (108 [128x128] units instead of 144), cutting score
  and AV matmul work by 25%. S^T[k,q] = kT.T @ qT over [c0:c1) (bf16 in,
  f32 psum); w = exp(S^T) bf16 on Act (AV deferred one chunk to hide exp
  latency); only the diagonal/window-edge 128x128 blocks get 0/1 masks
  (2 patterns, DVE); attnT accumulated via per-region PSUM writes (the
  first matmul's start=True pending-zeroes the whole bank; later
  matmuls split at the coverage boundary so each write region is
  uniformly fresh or accumulating); wsum += w per region (DVE);
  denom = ones.T @ wsum (one PE matmul, issued before the final AV so
  the DVE add/recip chain overlaps it) + exp(sink); 1/denom broadcast
  via GpSimd; attnT *= rbc.
o_proj in transposed layout: outT[oc, tok] += woT.T @ attnT, written
  bf16 to a packed DRAM tensor; host sums partials and transposes (free).
  For the last tile the first chunk group pre-runs its hd0-2 matmuls
  across four banks so the last head's softmax finalize overlaps o_proj.

Startup DMA: the dynamic HW queues race fairly for HBM, so transfers
are split to consumption order and later-needed tensors (wq tails,
cos/sin tails, masks, wo) are gated behind scalar-engine ops whose
data deps release them exactly when slot 0 stops needing the bandwidth.
h tiles live in 5 rotating buffers (tags (4n+q)%5) so each slot's first
h load is not WAR-blocked on the previous slot's Q-pass reads.

Softmax uses a constant (zero) max-shift: scores are bounded (|s| < ~10)
far below fp32/bf16 exp overflow, and softmax is shift-invariant; the
sink logit enters the denominator as exp(sink).
"""
import os
import numpy as np
import ml_dtypes

import concourse.mybir as mybir
import concourse.tile as tile
from concourse import bacc
from concourse.bass_utils import run_bass_kernel_spmd
from contextlib import ExitStack

F32 = mybir.dt.float32
BF16 = mybir.dt.bfloat16

S = 2048
HID = 4096
NQ = 32
NKV = 8
D = 128
WINDOW = 1024
THETA = 1e6
CORES = 8
QH = NQ // CORES          # 4 q heads per core
DQ = QH * D               # 512
NT = S // 512             # 4 token tiles of 512
KS = S // 128             # 16 key subtiles of 128

KT_DT = BF16

last_results = None       # set by kernel(); test.py reads exec_time_ns


def _schedule(positions):
    """Static attention schedule from the actual positions array.

    Returns (masks_np [128, P*128] bf16, sched, n_patterns) where
    sched[qt] = list of (ks, c0, c1, mask_ops); [c0,c1) is the visible
    query-column range within the tile (128-aligned) and mask_ops is a
    list of (col_offset, pidx) for partially-visible 128x128 blocks.
    """
    pos = np.asarray(positions).astype(np.int64)
    vis = (pos[None, :] <= pos[:, None]) & (pos[:, None] - pos[None, :] < WINDOW)
    patterns = {}
    plist = []
    sched = []
    for qt in range(NT):
        row = []
        for ks in range(KS):
            sub = vis[qt * 512:(qt + 1) * 512, ks * 128:(ks + 1) * 128]  # [q, k]
            qcols = sub.any(axis=1)
            if not qcols.any():
                continue
            c0 = int(np.argmax(qcols))
            c1 = 512 - int(np.argmax(qcols[::-1]))
            assert qcols[c0:c1].all(), "visible query range must be contiguous"
            c0 = (c0 // 128) * 128
            c1 = ((c1 + 127) // 128) * 128
            mops = []
            for u in range(c0, c1, 128):
                blk = sub[u:u + 128, :]
                if not blk.all():
                    pat = np.ascontiguousarray(blk.T).astype(np.float32)  # [k, q]
                    key = pat.tobytes()
                    if key not in patterns:
                        patterns[key] = len(plist)
                        plist.append(pat)
                    mops.append((u, patterns[key]))
            row.append((ks, c0, c1, mops))
        sched.append(row)
    if not plist:
        plist = [np.ones((128, 128), np.float32)]
    masks = np.concatenate(plist, axis=1).astype(ml_dtypes.bfloat16)  # [128, P*128]
    return masks, sched, len(plist)


def _build(sched, n_patterns):
    nc = bacc.Bacc("TRN2", target_bir_lowering=False)

    # Host-packed inputs: every load is a contiguous [128, n] slab.
    HB = nc.dram_tensor("hb", [128, NT * 4 * 8 * 512], BF16, kind="ExternalInput")
    WKV = nc.dram_tensor("wkv", [128, 2 * 32 * 128], BF16, kind="ExternalInput")
    WQa = nc.dram_tensor("wqa", [128, 32 * 256], BF16, kind="ExternalInput")
    WQb = nc.dram_tensor("wqb", [128, 32 * 256], BF16, kind="ExternalInput")
    WOT = nc.dram_tensor("wot", [128, QH * 32 * 128], BF16, kind="ExternalInput")
    Cos = nc.dram_tensor("cos", [128, S], F32, kind="ExternalInput")
    Sin = nc.dram_tensor("sin", [128, S], F32, kind="ExternalInput")
    Mk = nc.dram_tensor("mk", [128, n_patterns * 128], BF16, kind="ExternalInput")
    One = nc.dram_tensor("one", [128, 1], BF16, kind="ExternalInput")
    Esk = nc.dram_tensor("esk", [128, QH], F32, kind="ExternalInput")
    # packed partial output: cols = qt*16384 + o*512 + t  (o = hid/128 chunk)
    OutP = nc.dram_tensor("outp", [128, NT * 32 * 512], BF16, kind="ExternalOutput")

    Exp = mybir.ActivationFunctionType.Exp

    with tile.TileContext(nc) as tc, ExitStack() as top:
        persist = top.enter_context(tc.tile_pool(name="persist", bufs=1))
        ones = persist.tile([128, 1], BF16)
        esk = persist.tile([128, QH], F32)
        cos_sb = persist.tile([128, S], F32)
        sin_sb = persist.tile([128, S], F32)
        mk_sb = persist.tile([128, n_patterns * 128], BF16)
        wot_sb = persist.tile([128, QH * 32 * 128], BF16)
        wq_sb = [persist.tile([128, 32 * 256], BF16, tag=f"wq{i}", name=f"wq{i}")
                 for i in range(2)]
        wkv_sb = persist.tile([128, 2 * 32 * 128], BF16)
        # qT[m][n] is dead once b_stream(n) finishes (slot n+1), and
        # proj(n+2) rewrites the same parity only in slot n+2 — so two
        # buffers per head suffice (saves 8KB/partition of SBUF)
        qT_ = [[persist.tile([128, 512], BF16, tag=f"qT{m}_{p_}", name=f"qT{m}_{p_}")
                for p_ in range(2)] for m in range(QH)]
        qT = [[qT_[m][n % 2] for n in range(NT)] for m in range(QH)]
        kT = [persist.tile([128, 512], KT_DT, tag=f"kT{n}", name=f"kT{n}") for n in range(NT)]
        v_sb = [persist.tile([128, 512], BF16, tag=f"v{n}", name=f"v{n}") for n in range(NT)]

        def rope(dst, ps, n):
            co = cos_sb[:, n * 512:(n + 1) * 512]
            si = sin_sb[:, n * 512:(n + 1) * 512]
            t2 = rtmp.tile([128, 512], F32, tag="t2", name="t2")
            nc.vector.tensor_mul(t2[0:64, :], ps[64:128, :], si[0:64, :])
            nc.vector.tensor_mul(t2[64:128, :], ps[0:64, :], si[64:128, :])
            tc_ = rtmp.tile([128, 512], F32, tag="tc", name="tc")
            nc.vector.tensor_mul(tc_[:], ps[:], co)
            nc.vector.tensor_add(dst, tc_[:], t2[:])

        # ------- Slots: proj(n) [kv + q passes] + attn(n-1) + o_proj(n-2)
        with ExitStack() as pq:
            hqp = pq.enter_context(tc.tile_pool(name="hqp", bufs=1))
            rtmp = pq.enter_context(tc.tile_pool(name="rtmpq", bufs=2))
            vtp = pq.enter_context(tc.tile_pool(name="vtp", bufs=2))
            wpool = pq.enter_context(tc.tile_pool(name="wpool", bufs=7))
            wspool = pq.enter_context(tc.tile_pool(name="wspool", bufs=2))
            dpool = pq.enter_context(tc.tile_pool(name="dpool", bufs=2))
            apool = pq.enter_context(tc.tile_pool(name="apool", bufs=1))
            obp = pq.enter_context(tc.tile_pool(name="obp", bufs=2))
            ps_q = pq.enter_context(tc.tile_pool(name="ps_q", bufs=1, space="PSUM"))
            ps_s = pq.enter_context(tc.tile_pool(name="ps_s", bufs=2, space="PSUM"))
            ps_a = pq.enter_context(tc.tile_pool(name="ps_a", bufs=2, space="PSUM"))
            ps_o = pq.enter_context(tc.tile_pool(name="ps_o", bufs=2, space="PSUM"))

            attnT = [[None] * QH for _ in range(NT)]

            # ---- Startup DMA.  The dynamic HW queues race (fair-share
            # bandwidth), so prioritization = eager-issue only what slot 0
            # needs (h0 + wkv + cos0 + wqa ~ 8.5MB) and GATE everything else
            # behind the vt copy, whose KV-pass dependency releases it
            # exactly when the Q passes begin (see proj_stream).
            # h lives in 5 rotating whole tiles of [128, 4096] (tags
            # hq0-hq4); slot n uses tags (4n+q)%5 so each slot's first load
            # hits a fresh buffer and the rest WAR-release progressively as
            # the previous slot's Q passes retire their reads.
            def hq_tiles(n):
                out = []
                for q in range(4):
                    t_ = (4 * n + q) % 5
                    out.append(hqp.tile([128, 4096], BF16, tag=f"hq{t_}",
                                        name=f"hq{t_}"))
                return out

            hq0 = hq_tiles(0)
            # Anchor-laddered startup: the dynamic DMA queues share HBM
            # fairly, so an all-at-once issue makes everything arrive
            # together and late.  Instead each consumption stage's
            # transfers trigger behind a tiny scalar op that depends on the
            # previous h tile's arrival, keeping arrival order ~= use order
            # at near-full per-stage bandwidth.
            nc.sync.dma_start(hq0[0][:, 0:1024], HB[:, 0:1024])
            nc.sync.dma_start(hq0[0][:, 1024:2048], HB[:, 1024:2048])
            nc.sync.dma_start(hq0[0][:, 2048:4096], HB[:, 2048:4096])
            nc.sync.dma_start(cos_sb[:, 0:512], Cos[:, 0:512])
            nc.sync.dma_start(sin_sb[:, 0:512], Sin[:, 0:512])
            nc.scalar.dma_start(wkv_sb[:, 0:1024], WKV[:, 0:1024])
            nc.scalar.dma_start(wkv_sb[:, 4096:5120], WKV[:, 4096:5120])
            nc.gpsimd.dma_start(esk[:], Esk[:])
            nc.gpsimd.dma_start(ones[:], One[:])

            def ladder(stage, src_tile, anchor_col, dmas):
                # anchor on an early piece so the next stage joins the race
                # before the tile fully lands
                a_ = dpool.tile([1, 1], BF16, tag="dum", name=f"anc{stage}")
                nc.scalar.copy(a_[:], src_tile[0:1, anchor_col:anchor_col + 1])
                for dst, src in dmas:
                    nc.scalar.dma_start(dst, src)

            ladder(0, hq0[0], 2047, [
                (hq0[1][:, 0:2048], HB[:, 4096:6144]),
                (hq0[1][:, 2048:4096], HB[:, 6144:8192]),
                (wkv_sb[:, 1024:2048], WKV[:, 1024:2048]),
                (wkv_sb[:, 5120:6144], WKV[:, 5120:6144]),
                (wq_sb[0][:, 0:2048], WQa[:, 0:2048]),
            ])
            ladder(1, hq0[1], 2047, [
                (hq0[2][:, 0:2048], HB[:, 8192:10240]),
                (hq0[2][:, 2048:4096], HB[:, 10240:12288]),
                (wkv_sb[:, 2048:3072], WKV[:, 2048:3072]),
                (wkv_sb[:, 6144:7168], WKV[:, 6144:7168]),
                (wq_sb[0][:, 2048:4096], WQa[:, 2048:4096]),
            ])
            ladder(2, hq0[2], 2047, [
                (hq0[3][:, 0:2048], HB[:, 12288:14336]),
                (hq0[3][:, 2048:4096], HB[:, 14336:16384]),
                (wkv_sb[:, 3072:4096], WKV[:, 3072:4096]),
                (wkv_sb[:, 7168:8192], WKV[:, 7168:8192]),
                (wq_sb[0][:, 4096:8192], WQa[:, 4096:8192]),
            ])
            ladder(3, hq0[3], 2047, [
                (wq_sb[1][:, 0:2048], WQb[:, 0:2048]),
                (wq_sb[1][:, 2048:8192], WQb[:, 2048:8192]),
            ])

            hq_cache = {0: hq0}

            def qpass(n, mp, pool_tags):
                hqs = hq_cache[n]
                qps = [pool_tags[0].tile([128, 512], F32, tag=pool_tags[1][j],
                                         name=f"qps{j}")
                       for j in range(2)]
                for q in range(4):
                    for j in range(2):
                        for half in range(2):
                            for kk in range(half * 4, half * 4 + 4):
                                nc.tensor.matmul(
                                    qps[j][:],
                                    wq_sb[mp][:, (q * 8 + kk) * 256 + j * 128:
                                            (q * 8 + kk) * 256 + (j + 1) * 128],
                                    hqs[q][:, kk * 512:(kk + 1) * 512],
                                    start=(q == 0 and kk == 0),
                                    stop=(q == 3 and kk == 7))
                            yield
                for j in range(2):
                    rope(qT[mp * 2 + j][n][:], qps[j][:], n)
                    yield

            def proj_stream(n, mps=(0, 1)):
                if n == 0:
                    hqs = hq0
                    for _ in range(4):
                        yield
                else:
                    hqs = hq_tiles(n)
                    hq_cache[n] = hqs
                    for q in range(4):
                        nc.sync.dma_start(hqs[q][:],
                                          HB[:, (n * 4 + q) * 4096:(n * 4 + q + 1) * 4096])
                        yield

                def hcol(q, kk):
                    return hqs[q][:, kk * 512:(kk + 1) * 512]
                # pass 1: k and v projections
                kvps = [ps_q.tile([128, 512], F32, tag=f"proj{j}", name=f"proj{j}")
                        for j in range(2)]
                for q in range(4):
                    for m in range(2):
                        for half in range(2):
                            for kk in range(half * 4, half * 4 + 4):
                                nc.tensor.matmul(
                                    kvps[m][:],
                                    wkv_sb[:, m * 4096 + (q * 8 + kk) * 128:
                                           m * 4096 + (q * 8 + kk + 1) * 128],
                                    hcol(q, kk),
                                    start=(q == 0 and kk == 0), stop=(q == 3 and kk == 7))
                            yield
                rope(kT[n][:], kvps[0][:], n)
                yield
                vt = vtp.tile([128, 512], BF16, tag="vt", name=f"vt{n}")
                nc.scalar.copy(vt[:], kvps[1][:])
                if n == 0:
                    # gated prefetch: the scalar engine reaches these triggers
                    # only once the vt copy's KV-pass dependency clears, so
                    # none of it races the slot-0 critical loads
                    nc.scalar.dma_start(cos_sb[:, 512:2048], Cos[:, 512:2048])
                    nc.scalar.dma_start(sin_sb[:, 512:2048], Sin[:, 512:2048])
                    nc.scalar.dma_start(mk_sb[:], Mk[:])
                for t in range(4):
                    nc.sync.dma_start_transpose(v_sb[n][:, t * 128:(t + 1) * 128],
                                                vt[:, t * 128:(t + 1) * 128])
                yield
                # passes 2/3: q heads in pairs, reusing the same two banks
                for mp in mps:
                    yield from qpass(n, mp, (ps_q, ("proj0", "proj1")))

            def b_stream(qt):
                row = sched[qt]

                for hd in range(QH):
                    a_ps = ps_a.tile([128, 512], F32, tag="a")
                    wsum = wspool.tile([128, 512], BF16, tag="ws")
                    cov = 0       # wsum coverage: always [0, cov)
                    acov = [0]    # a_ps coverage (AV is deferred one chunk)

                    def do_av(w, ks, c0, c1, last):
                        vst = v_sb[ks // 4][:, (ks % 4) * 128:(ks % 4 + 1) * 128]
                        regs = []
                        if acov[0] > c0:
                            regs.append((c0, min(c1, acov[0]), False))
                        if c1 > acov[0]:
                            regs.append((max(c0, acov[0]), c1, acov[0] == 0))
                        for ri, (lo, hi, st) in enumerate(regs):
                            nc.tensor.matmul(
                                a_ps[:, lo:hi], vst, w[:, lo:hi],
                                start=st, stop=(last and ri == len(regs) - 1))
                        acov[0] = max(acov[0], c1)

                    pend = None  # (w, ks, c0, c1): AV matmul deferred one chunk
                    for i, (ks, c0, c1, mops) in enumerate(row):
                        s_ps = ps_s.tile([128, 512], F32, tag="s")
                        nc.tensor.matmul(
                            s_ps[:, c0:c1],
                            kT[ks // 4][:, (ks % 4) * 128:(ks % 4 + 1) * 128],
                            qT[hd][qt][:, c0:c1], start=True, stop=True)
                        if pend is not None:
                            do_av(*pend, last=False)
                        w = wpool.tile([128, 512], BF16, tag="w")
                        nc.scalar.activation(w[:, c0:c1], s_ps[:, c0:c1], Exp)
                        for (u, pidx) in mops:
                            nc.vector.tensor_mul(
                                w[:, u:u + 128], w[:, u:u + 128],
                                mk_sb[:, pidx * 128:(pidx + 1) * 128])
                        hi_old = min(c1, cov)
                        if hi_old > c0:
                            nc.vector.tensor_add(wsum[:, c0:hi_old],
                                                 wsum[:, c0:hi_old], w[:, c0:hi_old])
                        if c1 > cov:
                            nc.vector.tensor_copy(wsum[:, cov:c1], w[:, cov:c1])
                            cov = c1
                        pend = (w, ks, c0, c1)
                        yield
                    # denom first (needs only wsum) so the add/reciprocal run
                    # under the final AV matmul
                    d_ps = ps_s.tile([128, 512], F32, tag="s")
                    nc.tensor.matmul(d_ps[0:1, :], ones[:], wsum[:],
                                     start=True, stop=True)
                    do_av(*pend, last=True)
                    den = dpool.tile([1, 512], F32, tag="den")
                    nc.vector.tensor_scalar_add(den[:], d_ps[0:1, :], esk[0:1, hd:hd + 1])
                    rec = dpool.tile([1, 512], F32, tag="rec")
                    nc.vector.reciprocal_approx_fast(rec[:], den[:])
                    rbc = dpool.tile([128, 512], F32, tag="rbc")
                    nc.gpsimd.partition_broadcast(rbc[:], rec[:])
                    if qt == 0 and hd == 0:
                        # deferred bulk prefetch: the gpsimd engine reaches
                        # this trigger mid-slot-1, after the h1 race clears
                        nc.gpsimd.dma_start(wot_sb[:], WOT[:])
                    at = apool.tile([128, 512], BF16, tag=f"at{hd}_{qt % 2}")
                    nc.vector.tensor_mul(at[:], a_ps[:], rbc[:])
                    attnT[qt][hd] = at
                    yield

            def c_stream(qt):
                # the last tile runs with no other stream to hide the PSUM
                # drain, so borrow the idle proj banks for 4-deep pipelining
                deep = qt == NT - 1
                for og in range(8):
                    ob = obp.tile([128, 2048], BF16, tag="ob")
                    if deep and og == 0:
                        # prestart: run hd0-2 for SIX chunks (og0 c0-3 plus
                        # og1 c0-1, borrowing the idle ps_s/ps_a buffers)
                        # while the last head's softmax finalize (DVE/GpSimd
                        # chain, ~3.5us) produces attnT[qt][3].  The greedy
                        # scheduler drains independent PE work before the
                        # chain starts, so there must be more of it than the
                        # preceding score phase can absorb.
                        # the 7th chunk reuses d_ps's buffer: its WAR dep on
                        # the chain's denominator read makes it available
                        # only mid-chain, so the scheduler cannot hoist it
                        pre = []
                        for ci in range(7):
                            og_, c = divmod(ci, 4)
                            if ci in (4, 6):
                                o_ps = ps_s.tile([128, 512], F32, tag="s",
                                                 name="o_ps")
                            elif ci == 5:
                                o_ps = ps_a.tile([128, 512], F32, tag="a",
                                                 name="o_ps")
                            elif c % 2:
                                o_ps = ps_q.tile([128, 512], F32,
                                                 tag=f"proj{(c // 2) % 2}", name="o_ps")
                            else:
                                o_ps = ps_o.tile([128, 512], F32, tag="o")
                            pre.append(o_ps)
                            o = og_ * 4 + c
                            for hd in range(QH - 1):
                                nc.tensor.matmul(
                                    o_ps[:],
                                    wot_sb[:, (hd * 32 + o) * 128:(hd * 32 + o + 1) * 128],
                                    attnT[qt][hd][:],
                                    start=(hd == 0), stop=False)
                        yield
                        hd = QH - 1
                        for c in range(4):
                            nc.tensor.matmul(
                                pre[c][:],
                                wot_sb[:, (hd * 32 + c) * 128:(hd * 32 + c + 1) * 128],
                                attnT[qt][hd][:], start=False, stop=True)
                            if c % 2:
                                nc.scalar.copy(ob[:, c * 512:(c + 1) * 512], pre[c][:])
                            else:
                                nc.vector.tensor_copy(ob[:, c * 512:(c + 1) * 512],
                                                      pre[c][:])
                            yield
                        nc.gpsimd.dma_start(OutP[:, qt * 16384:qt * 16384 + 2048], ob[:])
                        yield
                        continue
                    if deep and og == 1:
                        for c in range(3):
                            o = 4 + c
                            nc.tensor.matmul(
                                pre[4 + c][:],
                                wot_sb[:, ((QH - 1) * 32 + o) * 128:
                                       ((QH - 1) * 32 + o + 1) * 128],
                                attnT[qt][QH - 1][:], start=False, stop=True)
                            if c % 2:
                                nc.scalar.copy(ob[:, c * 512:(c + 1) * 512], pre[4 + c][:])
                            else:
                                nc.vector.tensor_copy(ob[:, c * 512:(c + 1) * 512],
                                                      pre[4 + c][:])
                            yield
                        for c in range(3, 4):
                            o = og * 4 + c
                            o_ps = ps_q.tile([128, 512], F32, tag=f"proj{og % 2}",
                                             name="o_ps") if c % 2 else \
                                ps_o.tile([128, 512], F32, tag="o", name="o_ps")
                            for hd in range(QH):
                                nc.tensor.matmul(
                                    o_ps[:],
                                    wot_sb[:, (hd * 32 + o) * 128:(hd * 32 + o + 1) * 128],
                                    attnT[qt][hd][:],
                                    start=(hd == 0), stop=(hd == QH - 1))
                            if c % 2:
                                nc.scalar.copy(ob[:, c * 512:(c + 1) * 512], o_ps[:])
                            else:
                                nc.vector.tensor_copy(ob[:, c * 512:(c + 1) * 512], o_ps[:])
                            yield
                        nc.gpsimd.dma_start(
                            OutP[:, qt * 16384 + 2048:qt * 16384 + 4096], ob[:])
                        yield
                        continue
                    for c in range(4):
                        o = og * 4 + c
                        if deep and c % 2:
                            o_ps = ps_q.tile([128, 512], F32, tag=f"proj{og % 2}",
                                             name="o_ps")
                        else:
                            o_ps = ps_o.tile([128, 512], F32, tag="o")
                        for hd in range(QH):
                            nc.tensor.matmul(
                                o_ps[:],
                                wot_sb[:, (hd * 32 + o) * 128:(hd * 32 + o + 1) * 128],
                                attnT[qt][hd][:],
                                start=(hd == 0), stop=(hd == QH - 1))
                        if c % 2:
                            nc.scalar.copy(ob[:, c * 512:(c + 1) * 512], o_ps[:])
                        else:
                            nc.vector.tensor_copy(ob[:, c * 512:(c + 1) * 512], o_ps[:])
                        if deep and og == 7 and c == 1:
                            # start draining the final chunk group early so
                            # the tail DMA overlaps the last two chunks
                            nc.gpsimd.dma_start(
                                OutP[:, qt * 16384 + og * 2048:
                                     qt * 16384 + og * 2048 + 1024],
                                ob[:, 0:1024])
                        yield
                    if deep and og == 7:
                        nc.gpsimd.dma_start(
                            OutP[:, qt * 16384 + og * 2048 + 1024:
                                 qt * 16384 + (og + 1) * 2048],
                            ob[:, 1024:2048])
                    else:
                        nc.gpsimd.dma_start(
                            OutP[:, qt * 16384 + og * 2048:qt * 16384 + (og + 1) * 2048],
                            ob[:])
                    yield
                    if not deep:
                        yield  # stretch to pace the longer b_stream rows

            for slot in range(NT + 2):
                streams = []
                if slot == 0:
                    # slot 0 is DMA-paced: run KV and both Q passes as
                    # parallel streams on disjoint psum banks (ps_o/ps_s are
                    # idle here) so Q matmuls for already-arrived h tiles
                    # fill the KV pass's HBM stalls
                    streams.append(proj_stream(0, mps=()))
                    streams.append(qpass(0, 0, (ps_o, ("o", "o"))))
                    streams.append(qpass(0, 1, (ps_s, ("s", "s"))))
                elif slot < NT:
                    streams.append(proj_stream(slot))
                if 0 <= slot - 1 < NT:
                    streams.append(b_stream(slot - 1))
                if 0 <= slot - 2 < NT:
                    streams.append(c_stream(slot - 2))
                while streams:
                    alive = []
                    for st in streams:
                        try:
                            next(st)
                            alive.append(st)
                        except StopIteration:
                            pass
                    streams = alive

    nc.compile()
    return nc


def kernel(hidden_states, positions, wq, wk, wv, wo, sink):
    global last_results
    h = np.asarray(hidden_states, np.float32)
    pos = np.asarray(positions)
    wq = np.asarray(wq, np.float32)
    wk = np.asarray(wk, np.float32)
    wv = np.asarray(wv, np.float32)
    wo = np.asarray(wo, np.float32)
    sink = np.asarray(sink, np.float32)

    masks, sched, n_pat = _schedule(pos)
    nc = _build(sched, n_pat)

    # h packed: [p, (n, q, kt, t)] = h[n*512+t, (q*8+kt)*128+p]
    hp = np.ascontiguousarray(
        h.reshape(NT, 512, 4, 8, 128).transpose(4, 0, 2, 3, 1).reshape(128, -1)
    ).astype(ml_dtypes.bfloat16)

    # RoPE tables (neox half-split), rows duplicated for both halves
    inv_freq = 1.0 / (THETA ** (np.arange(0, D, 2, dtype=np.float64) / D))
    freqs = pos.astype(np.float64)[:, None] * inv_freq[None, :]       # [S, 64]
    cos = np.cos(freqs).astype(np.float32).T                          # [64, S]
    sin = np.sin(freqs).astype(np.float32).T
    cos_full = np.ascontiguousarray(np.concatenate([cos, cos], axis=0))
    sin_sign = np.ascontiguousarray(np.concatenate([-sin, sin], axis=0))

    scale = np.float32(D ** -0.5)
    esink = np.exp(sink.astype(np.float64)).astype(np.float32)

    in_maps = []
    for c in range(CORES):
        wqc = (wq[:, c * DQ:(c + 1) * DQ] * scale)                    # [HID, 512]
        # [p, kt, m*128+i] = wqc[kt*128+p, :], split into head pairs
        wqr = wqc.reshape(32, 128, 512).transpose(1, 0, 2)
        wqa = np.ascontiguousarray(wqr[:, :, 0:256].reshape(128, -1)).astype(ml_dtypes.bfloat16)
        wqb = np.ascontiguousarray(wqr[:, :, 256:512].reshape(128, -1)).astype(ml_dtypes.bfloat16)
        wkc = wk[:, c * D:(c + 1) * D].reshape(32, 128, 128).transpose(1, 0, 2)
        wvc = wv[:, c * D:(c + 1) * D].reshape(32, 128, 128).transpose(1, 0, 2)
        wkvp = np.ascontiguousarray(
            np.concatenate([wkc.reshape(128, -1), wvc.reshape(128, -1)], axis=1)
        ).astype(ml_dtypes.bfloat16)
        woc = wo[c * DQ:(c + 1) * DQ, :]                              # [512, HID]
        # [p, hd, o, i] = woc[hd*128+p, o*128+i]
        wotp = np.ascontiguousarray(
            woc.reshape(QH, 128, 32, 128).transpose(1, 0, 2, 3).reshape(128, -1)
        ).astype(ml_dtypes.bfloat16)
        in_maps.append({
            "hb": hp,
            "wqa": wqa,
            "wqb": wqb,
            "wkv": wkvp,
            "wot": wotp,
            "cos": cos_full,
            "sin": sin_sign,
            "mk": masks,
            "one": np.ones((128, 1), np.float32).astype(ml_dtypes.bfloat16),
            "esk": np.ascontiguousarray(
                np.broadcast_to(esink[None, c * QH:(c + 1) * QH], (128, QH)).copy()),
        })

    trace = bool(int(os.environ.get("KERNEL_TRACE", "0")))
    res = run_bass_kernel_spmd(nc, in_maps, core_ids=list(range(CORES)), trace=trace)
    last_results = res
    acc = np.zeros((128, NT * 32 * 512), np.float64)
    for r in res.results:
        acc += r["outp"].astype(np.float64)
    # [p, qt, o, t] -> out[qt*512+t, o*128+p]
    out = acc.reshape(128, NT, 32, 512).transpose(1, 3, 2, 0).reshape(S, HID)
    return out.astype(np.float32)
